# revision 37
# baseline (speedup 1.0000x reference)
"""MoE routed conv for Trainium2, 8-core SPMD.

Math: each batch image selects one expert (argmax of scores); the forward
output equals a 3x3 pad-1 conv of that image with the selected expert's
[128,128,3,3] filter (the dense conv + one-hot mask in the reference).
So we compute only the selected expert's conv: 5x less work.

Distribution: data-parallel over batch, 4 images per core. Host does the
(trivial) argmax routing + weight gather; the device program is uniform SPMD.

Device kernel (per core): shifted-window implicit GEMM in fp32r (TF32-like,
~2x faster than fp32 matmul in practice; the PE rounds operands internally).
  - Host ships x pre-padded to [128ci, 56h, 58w] (zero border columns), so
    loads are big contiguous HWDGE DMAs (small per-partition chunks collapse
    HBM read efficiency).
  - For each of 9 taps (kh,kw), one matmul per 8-row output chunk accumulates
    into PSUM: psum[co, h, w] += w_tap[ci,co].T @ xpad[ci, h+kh-1, w+kw].
  - Row clipping at image top/bottom shrinks the matmul row range; the PSUM
    destination stays full-width (fp32r dst runs must be 8B-aligned).
"""
import numpy as np

B, C, H, W = 32, 128, 56, 56
E, OC = 5, 128
NCORES = 8
IPC = B // NCORES          # images per core
CH = 8                     # output rows per chunk
NCHUNK = H // CH           # 7
WP = W + 2                 # padded width

_program = None


def _build_program():
    import concourse.bacc as bacc
    import concourse.tile as tile
    from concourse.tile import add_dep_helper
    from concourse import mybir

    dt = mybir.dt
    idt = dt.float32r
    nc = bacc.Bacc("TRN2", target_bir_lowering=False, debug=False)
    x_d = nc.dram_tensor("x", [IPC, C, H, WP], idt, kind="ExternalInput").ap()
    w_d = nc.dram_tensor("w", [IPC, C, 9, OC], idt, kind="ExternalInput").ap()
    o_d = nc.dram_tensor("o", [IPC, OC, H, W], dt.float32, kind="ExternalOutput").ap()

    NXT = 3  # x-tile ring depth

    with tile.TileContext(nc) as tc:
        with (
            tc.tile_pool(name="xp", bufs=1) as xp,
            tc.tile_pool(name="wpool", bufs=1) as wpool,
            tc.tile_pool(name="opool", bufs=1) as opool,
            tc.tile_pool(name="ps", bufs=8, space="PSUM") as psp,
        ):
            xts = [xp.tile([C, H, WP], idt, name=f"xt{i}") for i in range(NXT)]
            wts = [wpool.tile([C, 9, OC], idt, name=f"wt{i}") for i in range(IPC)]
            ots = [opool.tile([OC, H, W], dt.float32, name=f"ot{i}") for i in range(2)]

            anchor = None  # gates img>=1 prefetch DMAs off the head's critical path
            for img in range(IPC):
                xt = xts[img % NXT]
                wt = wts[img]
                ot = ots[img % 2]
                loads = []
                if img == 0:
                    # image 0: small first pieces start compute early; the rest
                    # in big contiguous chunks (small per-partition chunks
                    # wreck HBM read efficiency)
                    loads.append(nc.sync.dma_start(out=wt[:, 0:3, :],
                                                   in_=w_d[img, :, 0:3, :]))
                    loads.append(nc.sync.dma_start(out=wt[:, 3:9, :],
                                                   in_=w_d[img, :, 3:9, :]))
                    xsegs = [(0, 9), (9, 17), (17, 33), (33, 56)]
                else:
                    loads.append(nc.sync.dma_start(out=wt[:], in_=w_d[img]))
                    xsegs = [(0, 56)]
                for (ra, rb) in xsegs:
                    loads.append(nc.scalar.dma_start(
                        out=xt[:, ra:rb, :], in_=x_d[img, :, ra:rb, :]))
                if img >= 1 and anchor is not None:
                    for ld in loads:
                        add_dep_helper(ld.ins, anchor.ins, sync=True,
                                       reason="delay prefetch past head-critical DMAs")

                last_img = img == IPC - 1
                subchunks = [(c * CH, CH) for c in range(NCHUNK)]
                if last_img:
                    # split the final chunk so its first half's flush overlaps
                    # the second half's matmuls (shorter serial tail)
                    subchunks = subchunks[:-1] + [(48, 4), (52, 4)]
                for c, (r0, ch) in enumerate(subchunks):
                    ps = psp.tile([OC, ch, W], dt.float32, name=f"ps{img}_{c}", tag="ps")
                    for i, (kh, kw) in enumerate(
                        (kh, kw) for kh in range(3) for kw in range(3)
                    ):
                        hs = max(r0, 1 - kh)
                        he = min(r0 + ch, H + 1 - kh)
                        rhs = xt[:, hs + kh - 1 : he + kh - 1, kw : kw + W]
                        out = ps[:, hs - r0 : he - r0, :]
                        mm = nc.tensor.matmul(out, wt[:, kh * 3 + kw, :], rhs,
                                              start=(i == 0), stop=(i == 8))
                    if c == 0:
                        anchor = mm
                    if not last_img:
                        nc.vector.tensor_copy(ot[:, r0 : r0 + ch, :], ps[:])
                        if r0 + ch == 32:
                            nc.sync.dma_start(out=o_d[img, :, 0:32, :], in_=ot[:, 0:32, :])
                        elif r0 + ch == 56:
                            nc.sync.dma_start(out=o_d[img, :, 32:56, :], in_=ot[:, 32:56, :])
                    else:
                        # last image: flush per (sub)chunk to shorten the tail
                        nc.vector.tensor_copy(ot[:, r0 : r0 + ch, :], ps[:])
                        nc.sync.dma_start(out=o_d[img, :, r0 : r0 + ch, :],
                                          in_=ot[:, r0 : r0 + ch, :])
    nc.compile()
    return nc


def _get_program():
    global _program
    if _program is None:
        _program = _build_program()
    return _program


def kernel(x: np.ndarray, scores: np.ndarray, weight: np.ndarray,
           **run_kwargs) -> np.ndarray:
    from concourse.bass_utils import run_bass_kernel_spmd

    x = np.asarray(x, dtype=np.float32)
    scores = np.asarray(scores, dtype=np.float32)
    weight = np.asarray(weight, dtype=np.float32)

    expert = np.argmax(scores, axis=1)                       # [B]
    w_sel = weight.reshape(E, OC, C, 3, 3)[expert]           # [B, co, ci, kh, kw]
    # lhsT layout: [ci, tap, co]
    # fp32r: no host rounding needed (the PE rounds fp32r operands internally)
    w_lhsT = np.ascontiguousarray(w_sel.transpose(0, 2, 3, 4, 1).reshape(B, C, 9, OC))
    xpad = np.zeros((B, C, H, WP), np.float32)
    xpad[:, :, :, 1 : W + 1] = x

    nc = _get_program()
    in_maps = [
        {"x": xpad[k * IPC : (k + 1) * IPC], "w": w_lhsT[k * IPC : (k + 1) * IPC]}
        for k in range(NCORES)
    ]
    res = run_bass_kernel_spmd(nc, in_maps, list(range(NCORES)), **run_kwargs)
    out = np.concatenate([res.results[k]["o"] for k in range(NCORES)], axis=0)
    if run_kwargs:
        kernel.last_results = res
    return out.astype(np.float32)
